# revision 7
# baseline (speedup 1.0000x reference)
"""StyleGAN2-style modulated 3x3 conv layer on 8 TRN2 NeuronCores.

Math (per sample b):
    style = latent @ (fc_weight * LAT**-0.5).T + fc_bias            [CIN]
    w     = weight * style[None,:,None,None]                        [COUT,CIN,3,3]
    w     = w * rsqrt(sum(w*w, (1,2,3)) + EPS) * w_mul_conv
    out   = lrelu(conv2d(x, w, pad=1) + bias, 0.2)

Folded form used here (weights stay shared across the batch):
    x_mod[i]  = x[i] * style[i]
    conv      = conv2d(x_mod, weight)             (shared weight, bf16 matmuls)
    d[o]      = w_mul_conv * rsqrt(sum_i style[i]^2 * ssq[o,i] + EPS)
                with ssq[o,i] = sum_k weight[o,i,kh,kw]^2           (host precomp)
    out[o]    = lrelu(conv[o] * d[o] + bias[o], 0.2)

Sharding: data-parallel over batch B=8, one sample per core; conv/FC weights
replicated. The conv runs as shift-and-accumulate matmuls over a zero-padded
66x66 image kept flat in SBUF: for each of the 9 taps, rhs is the flat image
shifted by dh*66+dw, accumulated into one PSUM bank per output row-block.

reps>1 replicates the per-sample body (input DMA + full compute + output DMA)
with double-buffered tiles so consecutive reps pipeline — used only for
slope-timing on HW (per-rep time = steady-state kernel throughput).
"""

import numpy as np
import ml_dtypes

B, CIN, COUT, K, LAT, H, W = 8, 512, 512, 3, 512, 64, 64
EPS = 1e-8
W_MUL_FC = LAT**-0.5
W_MUL_CONV = (2.0**0.5) * (CIN * K * K) ** -0.5

P = 128
CI_T = CIN // P  # 4 input-channel tiles
CO_T = COUT // P  # 4 output-channel tiles
LA_T = LAT // P  # 4 latent tiles
WP = W + 2  # padded width (66)
HP = H + 2  # padded height (66)
G = 68  # flat guard columns on each side
XCOLS = G + HP * WP + G  # 4492
ROWS_PER_BLK = 7
# conv row-blocks, grouped so each group's blocks accumulate concurrently in
# distinct PSUM banks while one lhsT stays loaded: 8x7 rows, then 2x4 rows
BLOCK_GROUPS = [
    [(1 + ROWS_PER_BLK * i, ROWS_PER_BLK) for i in range(8)],
    [(57, 4), (61, 4)],
]
NMAX = ROWS_PER_BLK * WP  # 462 fp32 <= one PSUM bank

_COMPILED = {}


def _build_nc(reps=1):
    import concourse.bass as bass
    import concourse.mybir as mybir
    from concourse import bacc
    from concourse.tile import TileContext

    fp32 = mybir.dt.float32
    bf16 = mybir.dt.bfloat16
    AF = mybir.ActivationFunctionType
    ALU = mybir.AluOpType

    nc = bacc.Bacc("TRN2", target_bir_lowering=False, debug=False)

    xp_d = nc.dram_tensor("xp", [CI_T, P, XCOLS], bf16, kind="ExternalInput")
    lat_d = nc.dram_tensor("lat", [LAT], fp32, kind="ExternalInput")
    wt_d = nc.dram_tensor("wt", [CI_T, P, CO_T * 9 * P], bf16, kind="ExternalInput")
    fct_d = nc.dram_tensor("fct", [LA_T, P, CIN], fp32, kind="ExternalInput")
    ssq_d = nc.dram_tensor("ssq", [CI_T, P, COUT], fp32, kind="ExternalInput")
    fcb_d = nc.dram_tensor("fcb", [P, CI_T], fp32, kind="ExternalInput")
    cb_d = nc.dram_tensor("cbias", [P, CO_T], fp32, kind="ExternalInput")
    out_d = nc.dram_tensor("out", [COUT, H, W], fp32, kind="ExternalOutput")

    inv_wmc2 = 1.0 / (W_MUL_CONV * W_MUL_CONV)

    with TileContext(nc) as tc, tc.tile_pool(name="persist", bufs=1) as persist:
        def tile0(shape, dtype, name):
            return persist.tile(shape, dtype, tag=name, name=name)

        # --- constants / weights: DMA'd once ---
        fct = [tile0([P, CIN], fp32, f"fct{i}") for i in range(LA_T)]
        fcb = tile0([P, CI_T], fp32, "fcb")
        cb = tile0([P, CO_T], fp32, "cb")
        ssq = [tile0([P, COUT], fp32, f"ssq{i}") for i in range(CI_T)]
        wsb = [tile0([P, CO_T * 9 * P], bf16, f"wsb{i}") for i in range(CI_T)]

        for l in range(LA_T):
            nc.sync.dma_start(fct[l][:], fct_d[l])
        nc.sync.dma_start(fcb[:], fcb_d[:])
        nc.sync.dma_start(cb[:], cb_d[:])
        for ci in range(CI_T):
            nc.sync.dma_start(ssq[ci][:], ssq_d[ci])

        with (
            tc.tile_pool(name="xpool", bufs=2) as xpool,
            tc.tile_pool(name="spool", bufs=2) as spool,
            tc.tile_pool(name="pconv", bufs=8, space="PSUM") as pconv,
            tc.tile_pool(name="zpool", bufs=10) as zpool,
            tc.tile_pool(name="ypool", bufs=10) as ypool,
            tc.tile_pool(name="dtmp", bufs=2) as dpool,
        ):
            for rep in range(reps):
                # --- per-sample inputs ---
                latsb = spool.tile([P, LA_T], fp32, tag="latsb", name=f"latsb_{rep}")
                nc.sync.dma_start(latsb[:], lat_d[:].rearrange("(l p) -> p l", p=P))
                xmod = [
                    xpool.tile([P, XCOLS], bf16, tag=f"xmod{i}", name=f"xmod{i}_{rep}")
                    for i in range(CI_T)
                ]
                for ci in range(CI_T):
                    nc.sync.dma_start(xmod[ci][:], xp_d[ci])
                if rep == 0:
                    # weights co-chunk-major so co=0 matmuls can start earliest
                    for co in range(CO_T):
                        for ci in range(CI_T):
                            s = co * 9 * P
                            nc.sync.dma_start(
                                wsb[ci][:, s : s + 9 * P], wt_d[ci, :, s : s + 9 * P]
                            )

                # --- style = latent @ fcT (fp32 matvec), then modulate x ---
                style = [
                    spool.tile([P, 1], fp32, tag=f"style{i}", name=f"style{i}_{rep}")
                    for i in range(CI_T)
                ]
                style2 = [
                    spool.tile([P, 1], fp32, tag=f"style2_{i}", name=f"style2_{i}_{rep}")
                    for i in range(CI_T)
                ]
                dscale = [
                    spool.tile([P, 1], fp32, tag=f"dscale{i}", name=f"dscale{i}_{rep}")
                    for i in range(CO_T)
                ]
                for ci in range(CI_T):
                    ps = pconv.tile([P, NMAX], fp32, tag="ps_conv", name=f"ps_st{ci}_{rep}")[:, :1]
                    for l in range(LA_T):
                        nc.tensor.matmul(
                            ps[:],
                            lhsT=fct[l][:, ci * P : (ci + 1) * P],
                            rhs=latsb[:, l : l + 1],
                            start=(l == 0),
                            stop=(l == LA_T - 1),
                        )
                    nc.scalar.activation(
                        style[ci][:], ps[:], AF.Identity,
                        bias=fcb[:, ci : ci + 1], scale=W_MUL_FC,
                    )
                    nc.scalar.activation(
                        style2[ci][:], ps[:], AF.Square,
                        bias=fcb[:, ci : ci + 1], scale=W_MUL_FC,
                    )
                    nc.vector.tensor_scalar_mul(xmod[ci][:], xmod[ci][:], style[ci][:])

                # --- demod scale d[o] (fp32 matvec + sqrt + recip + 1 Newton) ---
                for co in range(CO_T):
                    ps = pconv.tile([P, NMAX], fp32, tag="ps_conv", name=f"ps_d{co}_{rep}")[:, :1]
                    for ci in range(CI_T):
                        nc.tensor.matmul(
                            ps[:],
                            lhsT=ssq[ci][:, co * P : (co + 1) * P],
                            rhs=style2[ci][:],
                            start=(ci == 0),
                            stop=(ci == CI_T - 1),
                        )
                    sarg = dpool.tile([P, 1], fp32, tag="sarg", name=f"sarg{co}_{rep}")
                    sq = dpool.tile([P, 1], fp32, tag="sq", name=f"sq{co}_{rep}")
                    y0 = dpool.tile([P, 1], fp32, tag="y0", name=f"y0_{co}_{rep}")
                    u = dpool.tile([P, 1], fp32, tag="u", name=f"u{co}_{rep}")
                    v = dpool.tile([P, 1], fp32, tag="v", name=f"v{co}_{rep}")
                    # sarg = s / wmc^2 ; target d = 1/sqrt(sarg). EPS=1e-8 vs
                    # s ~ O(1e3) is ~1e-11 relative - dropped (no const-AP).
                    nc.scalar.activation(
                        sarg[:], ps[:], AF.Identity, bias=0.0, scale=inv_wmc2
                    )
                    nc.scalar.activation(
                        sq[:], ps[:], AF.Sqrt, bias=0.0, scale=inv_wmc2
                    )
                    nc.vector.reciprocal(y0[:], sq[:])
                    # Newton: y1 = y0*(1.5 - 0.5*sarg*y0^2) — ACT sqrt is low-ULP
                    nc.vector.tensor_mul(u[:], y0[:], y0[:])
                    nc.vector.tensor_mul(v[:], u[:], sarg[:])
                    nc.vector.tensor_scalar(
                        v[:], v[:], -0.5, 1.5, op0=ALU.mult, op1=ALU.add
                    )
                    nc.vector.tensor_mul(dscale[co][:], y0[:], v[:])

                # --- main conv: per co-tile, rotate each lhsT across the
                # group's PSUM banks (weight loads amortized over the group) ---
                for co in range(CO_T):
                    for gi, group in enumerate(BLOCK_GROUPS):
                        tiles = [
                            pconv.tile(
                                [P, NMAX], fp32, tag="ps_conv",
                                name=f"pc{co}_{gi}_{j}_{rep}",
                            )
                            for j in range(len(group))
                        ]
                        for kpos in range(9):
                            dh, dw = kpos // 3 - 1, kpos % 3 - 1
                            first = kpos == 0
                            last = kpos == 8
                            lhsT = None
                            for ci in range(CI_T):
                                lhsT = wsb[ci][
                                    :, (co * 9 + kpos) * P : (co * 9 + kpos + 1) * P
                                ]
                                for j, (r0, nr) in enumerate(group):
                                    N = nr * WP
                                    base = G + WP * (r0 + dh) + dw
                                    nc.tensor.matmul(
                                        tiles[j][:, :N],
                                        lhsT=lhsT,
                                        rhs=xmod[ci][:, base : base + N],
                                        start=(first and ci == 0),
                                        stop=(last and ci == CI_T - 1),
                                    )
                        for j, (r0, nr) in enumerate(group):
                            N = nr * WP
                            z = zpool.tile(
                                [P, NMAX], fp32, tag="z", name=f"z{co}_{gi}_{j}_{rep}"
                            )
                            yt = ypool.tile(
                                [P, NMAX], fp32, tag="y", name=f"y{co}_{gi}_{j}_{rep}"
                            )
                            nc.scalar.activation(
                                z[:, :N], tiles[j][:, :N], AF.Identity,
                                bias=cb[:, co : co + 1], scale=dscale[co][:],
                            )
                            nc.vector.scalar_tensor_tensor(
                                yt[:, :N], z[:, :N], 0.2, z[:, :N],
                                op0=ALU.mult, op1=ALU.max,
                            )
                            nc.sync.dma_start(
                                out_d[co * P : (co + 1) * P, r0 - 1 : r0 - 1 + nr, :],
                                yt[:, :N].rearrange("p (r w) -> p r w", w=WP)[
                                    :, :, 1 : 1 + W
                                ],
                            )

    nc.compile()
    return nc


def _get_compiled(reps=1):
    if reps not in _COMPILED:
        _COMPILED[reps] = _build_nc(reps)
    return _COMPILED[reps]


def _prep_inputs(x, latent, weight, bias, fc_weight, fc_bias):
    """Host-side layout preprocessing (no model FLOPs besides ssq reduction)."""
    bf = ml_dtypes.bfloat16
    # padded flat image per sample: [B, CI_T, P, XCOLS] bf16
    xpad = np.zeros((B, CIN, XCOLS), np.float32)
    xp66 = np.pad(x, ((0, 0), (0, 0), (1, 1), (1, 1))).reshape(B, CIN, HP * WP)
    xpad[:, :, G : G + HP * WP] = xp66
    xp = np.ascontiguousarray(xpad.reshape(B, CI_T, P, XCOLS)).astype(bf)

    # weights as lhsT tiles: wt[ci, p, (co*9+kpos)*P + m] = weight[co*P+m, ci*P+p, kh, kw]
    w6 = weight.reshape(CO_T, P, CI_T, P, 9)  # [co, m, ci, p, kpos]
    wt = np.ascontiguousarray(w6.transpose(2, 3, 0, 4, 1)).reshape(
        CI_T, P, CO_T * 9 * P
    ).astype(bf)

    fct = np.ascontiguousarray(fc_weight.T).reshape(LA_T, P, CIN).astype(np.float32)
    ssq = np.ascontiguousarray(
        (weight.astype(np.float64) ** 2).sum(axis=(2, 3)).T
    ).reshape(CI_T, P, COUT).astype(np.float32)
    fcb = np.ascontiguousarray(fc_bias.reshape(CI_T, P).T).astype(np.float32)
    cb = np.ascontiguousarray(bias.reshape(CO_T, P).T).astype(np.float32)
    lat = np.ascontiguousarray(latent).astype(np.float32)

    in_maps = []
    for b in range(B):
        in_maps.append(
            {
                "xp": xp[b],
                "lat": lat[b],
                "wt": wt,
                "fct": fct,
                "ssq": ssq,
                "fcb": fcb,
                "cbias": cb,
            }
        )
    return in_maps


def kernel(x, latent, weight, bias, fc_weight, fc_bias):
    from concourse.bass_utils import run_bass_kernel_spmd

    x = np.asarray(x, np.float32)
    latent = np.asarray(latent, np.float32)
    weight = np.asarray(weight, np.float32)
    bias = np.asarray(bias, np.float32)
    fc_weight = np.asarray(fc_weight, np.float32)
    fc_bias = np.asarray(fc_bias, np.float32)

    nc = _get_compiled()
    in_maps = _prep_inputs(x, latent, weight, bias, fc_weight, fc_bias)
    res = run_bass_kernel_spmd(nc, in_maps, core_ids=list(range(B)))
    out = np.stack([res.results[b]["out"] for b in range(B)], axis=0)
    return out.astype(np.float32)


# revision 8
# speedup vs baseline: 1.3821x; 1.3821x over previous
"""StyleGAN2-style modulated 3x3 conv layer on 8 TRN2 NeuronCores.

Math (per sample b):
    style = latent @ (fc_weight * LAT**-0.5).T + fc_bias            [CIN]
    w     = weight * style[None,:,None,None]                        [COUT,CIN,3,3]
    w     = w * rsqrt(sum(w*w, (1,2,3)) + EPS) * w_mul_conv
    out   = lrelu(conv2d(x, w, pad=1) + bias, 0.2)

Folded form used here (weights stay shared across the batch):
    x_mod[i]  = x[i] * style[i]
    conv      = conv2d(x_mod, weight)             (shared weight, bf16 matmuls)
    d[o]      = w_mul_conv * rsqrt(sum_i style[i]^2 * ssq[o,i] + EPS)
                with ssq[o,i] = sum_k weight[o,i,kh,kw]^2           (host precomp)
    out[o]    = lrelu(conv[o] * d[o] + bias[o], 0.2)

Sharding: data-parallel over batch B=8, one sample per core; conv/FC weights
replicated. The conv uses width-direction Winograd F(2,3): weights are
G-transformed on host into 12 [CIN,COUT] matrices (3 kh taps x 4 g), the
modulated zero-padded 66x66 image is B^T-transformed on-device (DVE) into 4
V planes of 66x32 tiles, and the height direction stays a direct 3-tap
accumulation via row shifts of the V planes. Each (co-tile, 16-row block)
accumulates 4 PSUM banks (one per g, 12 matmuls each); the output transform
Y0=m0+m1+m2 / Y1=m1-m2-m3 runs on DVE with demod scale folded into the PSUM
evacuation, then leaky-relu and an interleaved store. This streams 1.55x
fewer PE columns than direct 3x3 shift-and-accumulate (393K vs 608K cols).

reps>1 replicates the per-sample body (input DMA + full compute + output DMA)
with double-buffered tiles so consecutive reps pipeline — used only for
slope-timing on HW (per-rep time = steady-state kernel throughput).
"""

import numpy as np
import ml_dtypes

B, CIN, COUT, K, LAT, H, W = 8, 512, 512, 3, 512, 64, 64
EPS = 1e-8
W_MUL_FC = LAT**-0.5
W_MUL_CONV = (2.0**0.5) * (CIN * K * K) ** -0.5

P = 128
CI_T = CIN // P  # 4 input-channel tiles
CO_T = COUT // P  # 4 output-channel tiles
LA_T = LAT // P  # 4 latent tiles
WP = W + 2  # padded width (66)
HP = H + 2  # padded height (66)
G = 68  # flat guard columns on each side
XCOLS = G + HP * WP + G  # 4492
NJ = W // 2  # 32 winograd tiles per row (F(2,3) along width)
VCOLS = HP * NJ  # 66 rows x 32 tiles per V plane
NB = 4  # row-blocks per (co, g): 16 rows x 32 = 512 cols = one PSUM bank
NMAX = 512

_COMPILED = {}


def _build_nc(reps=1):
    import concourse.bass as bass
    import concourse.mybir as mybir
    from concourse import bacc
    from concourse.tile import TileContext

    fp32 = mybir.dt.float32
    bf16 = mybir.dt.bfloat16
    AF = mybir.ActivationFunctionType
    ALU = mybir.AluOpType

    nc = bacc.Bacc("TRN2", target_bir_lowering=False, debug=False)

    xp_d = nc.dram_tensor("xp", [CI_T, P, XCOLS], bf16, kind="ExternalInput")
    lat_d = nc.dram_tensor("lat", [LAT], fp32, kind="ExternalInput")
    wt_d = nc.dram_tensor("wt", [CI_T, P, CO_T * 12 * P], bf16, kind="ExternalInput")
    fct_d = nc.dram_tensor("fct", [LA_T, P, CIN], fp32, kind="ExternalInput")
    ssq_d = nc.dram_tensor("ssq", [CI_T, P, COUT], fp32, kind="ExternalInput")
    fcb_d = nc.dram_tensor("fcb", [P, CI_T], fp32, kind="ExternalInput")
    cb_d = nc.dram_tensor("cbias", [P, CO_T], fp32, kind="ExternalInput")
    out_d = nc.dram_tensor("out", [COUT, H, W], fp32, kind="ExternalOutput")

    inv_wmc2 = 1.0 / (W_MUL_CONV * W_MUL_CONV)

    with TileContext(nc) as tc, tc.tile_pool(name="persist", bufs=1) as persist:
        def tile0(shape, dtype, name):
            return persist.tile(shape, dtype, tag=name, name=name)

        # --- constants / weights: DMA'd once ---
        fct = [tile0([P, CIN], fp32, f"fct{i}") for i in range(LA_T)]
        fcb = tile0([P, CI_T], fp32, "fcb")
        cb = tile0([P, CO_T], fp32, "cb")
        ssq = [tile0([P, COUT], fp32, f"ssq{i}") for i in range(CI_T)]
        wsb = [tile0([P, CO_T * 12 * P], bf16, f"wsb{i}") for i in range(CI_T)]

        for l in range(LA_T):
            nc.sync.dma_start(fct[l][:], fct_d[l])
        nc.sync.dma_start(fcb[:], fcb_d[:])
        nc.sync.dma_start(cb[:], cb_d[:])
        for ci in range(CI_T):
            nc.sync.dma_start(ssq[ci][:], ssq_d[ci])

        with (
            tc.tile_pool(name="xpool", bufs=1) as xpool,
            tc.tile_pool(name="vpool", bufs=1) as vpool,
            tc.tile_pool(name="mpool", bufs=2) as mpool,
            tc.tile_pool(name="spool", bufs=2) as spool,
            tc.tile_pool(name="pconv", bufs=8, space="PSUM") as pconv,
            tc.tile_pool(name="ypool", bufs=2) as ypool,
            tc.tile_pool(name="dtmp", bufs=2) as dpool,
        ):
            for rep in range(reps):
                # --- per-sample inputs ---
                latsb = spool.tile([P, LA_T], fp32, tag="latsb", name=f"latsb_{rep}")
                nc.sync.dma_start(latsb[:], lat_d[:].rearrange("(l p) -> p l", p=P))
                xmod = [
                    xpool.tile([P, XCOLS], bf16, tag=f"xmod{i}", name=f"xmod{i}_{rep}")
                    for i in range(CI_T)
                ]
                for ci in range(CI_T):
                    nc.sync.dma_start(xmod[ci][:], xp_d[ci])
                if rep == 0:
                    # weights co-chunk-major so co=0 matmuls can start earliest
                    for co in range(CO_T):
                        for ci in range(CI_T):
                            s = co * 12 * P
                            nc.sync.dma_start(
                                wsb[ci][:, s : s + 12 * P], wt_d[ci, :, s : s + 12 * P]
                            )

                # --- style = latent @ fcT (fp32 matvec), then modulate x ---
                style = [
                    spool.tile([P, 1], fp32, tag=f"style{i}", name=f"style{i}_{rep}")
                    for i in range(CI_T)
                ]
                style2 = [
                    spool.tile([P, 1], fp32, tag=f"style2_{i}", name=f"style2_{i}_{rep}")
                    for i in range(CI_T)
                ]
                dscale = [
                    spool.tile([P, 1], fp32, tag=f"dscale{i}", name=f"dscale{i}_{rep}")
                    for i in range(CO_T)
                ]
                for ci in range(CI_T):
                    ps = pconv.tile([P, NMAX], fp32, tag="ps_conv", name=f"ps_st{ci}_{rep}")[:, :1]
                    for l in range(LA_T):
                        nc.tensor.matmul(
                            ps[:],
                            lhsT=fct[l][:, ci * P : (ci + 1) * P],
                            rhs=latsb[:, l : l + 1],
                            start=(l == 0),
                            stop=(l == LA_T - 1),
                        )
                    nc.scalar.activation(
                        style[ci][:], ps[:], AF.Identity,
                        bias=fcb[:, ci : ci + 1], scale=W_MUL_FC,
                    )
                    nc.scalar.activation(
                        style2[ci][:], ps[:], AF.Square,
                        bias=fcb[:, ci : ci + 1], scale=W_MUL_FC,
                    )
                    nc.vector.tensor_scalar_mul(xmod[ci][:], xmod[ci][:], style[ci][:])

                # --- demod scale d[o] (fp32 matvec + sqrt + recip + 1 Newton) ---
                for co in range(CO_T):
                    ps = pconv.tile([P, NMAX], fp32, tag="ps_conv", name=f"ps_d{co}_{rep}")[:, :1]
                    for ci in range(CI_T):
                        nc.tensor.matmul(
                            ps[:],
                            lhsT=ssq[ci][:, co * P : (co + 1) * P],
                            rhs=style2[ci][:],
                            start=(ci == 0),
                            stop=(ci == CI_T - 1),
                        )
                    sarg = dpool.tile([P, 1], fp32, tag="sarg", name=f"sarg{co}_{rep}")
                    sq = dpool.tile([P, 1], fp32, tag="sq", name=f"sq{co}_{rep}")
                    y0 = dpool.tile([P, 1], fp32, tag="y0", name=f"y0_{co}_{rep}")
                    u = dpool.tile([P, 1], fp32, tag="u", name=f"u{co}_{rep}")
                    v = dpool.tile([P, 1], fp32, tag="v", name=f"v{co}_{rep}")
                    # sarg = s / wmc^2 ; target d = 1/sqrt(sarg). EPS=1e-8 vs
                    # s ~ O(1e3) is ~1e-11 relative - dropped (no const-AP).
                    nc.scalar.activation(
                        sarg[:], ps[:], AF.Identity, bias=0.0, scale=inv_wmc2
                    )
                    nc.scalar.activation(
                        sq[:], ps[:], AF.Sqrt, bias=0.0, scale=inv_wmc2
                    )
                    nc.vector.reciprocal(y0[:], sq[:])
                    # Newton: y1 = y0*(1.5 - 0.5*sarg*y0^2) — ACT sqrt is low-ULP
                    nc.vector.tensor_mul(u[:], y0[:], y0[:])
                    nc.vector.tensor_mul(v[:], u[:], sarg[:])
                    nc.vector.tensor_scalar(
                        v[:], v[:], -0.5, 1.5, op0=ALU.mult, op1=ALU.add
                    )
                    nc.vector.tensor_mul(dscale[co][:], y0[:], v[:])

                # --- width-direction Winograd F(2,3): V[g] = B^T-combos of
                # modulated image columns (tile j covers padded cols 2j..2j+3);
                # height direction stays direct 3-tap via row shifts of V ---
                V = [
                    [
                        vpool.tile([P, VCOLS], bf16, tag=f"V{g}_{ci}", name=f"V{g}_{ci}_{rep}")
                        for ci in range(CI_T)
                    ]
                    for g in range(4)
                ]
                for g in range(4):
                    for ci in range(CI_T):
                        xm3 = xmod[ci][:, G : G + HP * WP].rearrange(
                            "p (h w) -> p h w", w=WP
                        )

                        def dphase(a):
                            if a % 2 == 0:
                                return xm3[:, :, a : a + 64].rearrange(
                                    "p h (j t) -> p h j t", t=2
                                )[:, :, :, 0]
                            return xm3[:, :, a - 1 : a + 63].rearrange(
                                "p h (j t) -> p h j t", t=2
                            )[:, :, :, 1]

                        vt = V[g][ci][:].rearrange("p (h j) -> p h j", j=NJ)
                        if g == 0:
                            nc.vector.tensor_sub(vt, dphase(0), dphase(2))
                        elif g == 1:
                            nc.vector.tensor_add(vt, dphase(1), dphase(2))
                        elif g == 2:
                            nc.vector.tensor_sub(vt, dphase(2), dphase(1))
                        else:
                            nc.vector.tensor_sub(vt, dphase(1), dphase(3))

                # --- conv: M[g] = sum_kh sum_ci U[kh,g] @ V[g] row-shifted;
                # 4 PSUM banks (one per g) per 16-row block, then the output
                # transform Y0 = m0+m1+m2, Y1 = m1-m2-m3 interleaves to w ---
                for co in range(CO_T):
                    for b in range(NB):
                        msb = []
                        pst = []
                        for g in range(4):
                            ps = pconv.tile(
                                [P, NMAX], fp32, tag="ps_conv", name=f"pc{co}_{b}_{g}_{rep}"
                            )
                            pst.append(ps)
                            idx = 0
                            # ci-outer: the group's first matmuls need only
                            # V[g][0], so PE starts before later transforms land
                            for ci in range(CI_T):
                                for kh in range(3):
                                    off = (16 * b + kh) * NJ
                                    nc.tensor.matmul(
                                        ps[:],
                                        lhsT=wsb[ci][
                                            :,
                                            ((co * 3 + kh) * 4 + g) * P : ((co * 3 + kh) * 4 + g + 1) * P,
                                        ],
                                        rhs=V[g][ci][:, off : off + NMAX],
                                        start=(idx == 0),
                                        stop=(idx == 11),
                                    )
                                    idx += 1
                            m = mpool.tile(
                                [P, NMAX], fp32, tag=f"m{g}", name=f"m{g}_{co}_{b}_{rep}"
                            )
                            # demod scale folds in here (distributes over the sum)
                            nc.scalar.activation(
                                m[:], ps[:], AF.Identity, bias=0.0, scale=dscale[co][:]
                            )
                            msb.append(m)
                        t0 = ypool.tile([P, NMAX], fp32, tag="t0", name=f"t0_{co}_{b}_{rep}")
                        t1 = ypool.tile([P, NMAX], fp32, tag="t1", name=f"t1_{co}_{b}_{rep}")
                        yt = ypool.tile([P, 2 * NMAX], fp32, tag="yt", name=f"yt_{co}_{b}_{rep}")
                        ytv = yt[:].rearrange("p (r j t) -> p r j t", j=NJ, t=2)
                        nc.vector.tensor_add(t0[:], msb[0][:], msb[1][:])
                        nc.vector.tensor_sub(t1[:], msb[1][:], msb[2][:])
                        # Y0 = t0 + bias + m2 -> even w ; Y1 = t1 + bias - m3 -> odd w
                        nc.vector.scalar_tensor_tensor(
                            ytv[:, :, :, 0].rearrange("p r j -> p (r j)"),
                            t0[:], cb[:, co : co + 1], msb[2][:],
                            op0=ALU.add, op1=ALU.add,
                        )
                        nc.vector.scalar_tensor_tensor(
                            ytv[:, :, :, 1].rearrange("p r j -> p (r j)"),
                            t1[:], cb[:, co : co + 1], msb[3][:],
                            op0=ALU.add, op1=ALU.subtract,
                        )
                        nc.vector.scalar_tensor_tensor(
                            yt[:], yt[:], 0.2, yt[:], op0=ALU.mult, op1=ALU.max
                        )
                        nc.sync.dma_start(
                            out_d[co * P : (co + 1) * P, 16 * b : 16 * b + 16, :],
                            yt[:].rearrange("p (r w) -> p r w", w=W),
                        )

    nc.compile()
    return nc


def _get_compiled(reps=1):
    if reps not in _COMPILED:
        _COMPILED[reps] = _build_nc(reps)
    return _COMPILED[reps]


def _prep_inputs(x, latent, weight, bias, fc_weight, fc_bias):
    """Host-side layout preprocessing (no model FLOPs besides ssq reduction)."""
    bf = ml_dtypes.bfloat16
    # padded flat image per sample: [B, CI_T, P, XCOLS] bf16
    xpad = np.zeros((B, CIN, XCOLS), np.float32)
    xp66 = np.pad(x, ((0, 0), (0, 0), (1, 1), (1, 1))).reshape(B, CIN, HP * WP)
    xpad[:, :, G : G + HP * WP] = xp66
    xp = np.ascontiguousarray(xpad.reshape(B, CI_T, P, XCOLS)).astype(bf)

    # width-Winograd weight transform U = G w (G for F(2,3)), as lhsT tiles:
    # wt[ci, p, ((co*3+kh)*4+g)*P + m] = U_g(weight[co*P+m, ci*P+p, kh, :])
    w6 = weight.reshape(CO_T, P, CI_T, P, 3, 3)  # [co, m, ci, p, kh, kw]
    U = np.stack(
        [
            w6[..., 0],
            (w6[..., 0] + w6[..., 1] + w6[..., 2]) * 0.5,
            (w6[..., 0] - w6[..., 1] + w6[..., 2]) * 0.5,
            w6[..., 2],
        ],
        axis=-1,
    )  # [co, m, ci, p, kh, g]
    wt = np.ascontiguousarray(U.transpose(2, 3, 0, 4, 5, 1)).reshape(
        CI_T, P, CO_T * 12 * P
    ).astype(bf)

    fct = np.ascontiguousarray(fc_weight.T).reshape(LA_T, P, CIN).astype(np.float32)
    ssq = np.ascontiguousarray(
        (weight.astype(np.float64) ** 2).sum(axis=(2, 3)).T
    ).reshape(CI_T, P, COUT).astype(np.float32)
    fcb = np.ascontiguousarray(fc_bias.reshape(CI_T, P).T).astype(np.float32)
    cb = np.ascontiguousarray(bias.reshape(CO_T, P).T).astype(np.float32)
    lat = np.ascontiguousarray(latent).astype(np.float32)

    in_maps = []
    for b in range(B):
        in_maps.append(
            {
                "xp": xp[b],
                "lat": lat[b],
                "wt": wt,
                "fct": fct,
                "ssq": ssq,
                "fcb": fcb,
                "cbias": cb,
            }
        )
    return in_maps


def kernel(x, latent, weight, bias, fc_weight, fc_bias):
    from concourse.bass_utils import run_bass_kernel_spmd

    x = np.asarray(x, np.float32)
    latent = np.asarray(latent, np.float32)
    weight = np.asarray(weight, np.float32)
    bias = np.asarray(bias, np.float32)
    fc_weight = np.asarray(fc_weight, np.float32)
    fc_bias = np.asarray(fc_bias, np.float32)

    nc = _get_compiled()
    in_maps = _prep_inputs(x, latent, weight, bias, fc_weight, fc_bias)
    res = run_bass_kernel_spmd(nc, in_maps, core_ids=list(range(B)))
    out = np.stack([res.results[b]["out"] for b in range(B)], axis=0)
    return out.astype(np.float32)
